# revision 17
# baseline (speedup 1.0000x reference)
"""Trainium2 Bass kernel for gated-adapter attention (Llama-Adapter style).

Sharding: 2 data-parallel groups of 4 cores (batch dim); within a group the 32
heads are tensor-parallel (8 heads/core).  Each core computes QKV + RoPE +
causal attention (transposed scores) + gated adapter cross attention for its
heads, AllGathers the per-head attention output across its group of 4, then
computes a column shard of the wo projection.  The host reassembles the full
[2, 2048, 4096] output from the 8 per-core shards.

v2 restructure (trace-driven):
 - softmax denominators via fp16 pair-tree on DVE + one ones-matmul per
   (head, qblock) instead of one ones-matmul per key chunk (PE -150us).
 - exp computed on [128,1024] chunk pairs (halves ACT overhead).
 - causal mask via a single 128x128 triangle tile + zero-fill (no mask DMAs).
 - projections run m-outer over [128,256] x tiles (bigger DMA lines, tiny
   x pool); weight DMAs interleaved with first x tiles so PE starts early.
 - q blocks kept in SBUF; wo weights prefetched under attention; wo follows
   immediately after the last attention block.
"""

import math
import os
import sys

import numpy as np

for _p in ("/opt/trn_rl_repo",):
    if os.path.isdir(_p) and _p not in sys.path:
        sys.path.insert(0, _p)

import ml_dtypes  # noqa: E402

import concourse.bass as bass  # noqa: E402
import concourse.mybir as mybir  # noqa: E402
import concourse.tile as tile  # noqa: E402
from concourse import bacc  # noqa: E402
from concourse import bass_isa  # noqa: E402

FP16 = np.float16
F16 = mybir.dt.float16
F32 = mybir.dt.float32

DIM = 4096
S = 2048
B = 2
H = 32
HD = 128
ALEN = 10

NCORES = 8
CPG = 4          # cores per group (group = one batch element)
HPC = 8          # heads per core
OC = HPC * HD    # 1024 output dims per core for q/k/v and for wo columns

TCN = 17         # t-chunks of 128: 16 real + 1 adapter/pad chunk
TAUG = TCN * 128  # 2176
NM = DIM // 128   # 32 contraction chunks
QB = 4           # query blocks
QW = 512         # query block width
SCALE = 1.0 / math.sqrt(HD)

REPLICA_GROUPS = [[0, 1, 2, 3], [4, 5, 6, 7]]

Exp = mybir.ActivationFunctionType.Exp
Copy = mybir.ActivationFunctionType.Copy


def _alu():
    from concourse.alu_op_type import AluOpType
    return AluOpType


def build_graph():
    nc = bacc.Bacc(
        "TRN2",
        target_bir_lowering=False,
        debug=False,
        num_devices=NCORES,
    )
    op = _alu()

    # ---- external I/O ------------------------------------------------------
    xT = nc.dram_tensor("xT", [DIM, TAUG], F16, kind="ExternalInput")
    wqT = nc.dram_tensor("wqT", [DIM, OC], F16, kind="ExternalInput")
    wkT = nc.dram_tensor("wkT", [DIM, OC], F16, kind="ExternalInput")
    wvT = nc.dram_tensor("wvT", [DIM, OC], F16, kind="ExternalInput")
    woT = nc.dram_tensor("woT", [DIM, OC], F16, kind="ExternalInput")
    cosS = nc.dram_tensor("cosS", [128, 16 * 64], F16, kind="ExternalInput")
    sinS = nc.dram_tensor("sinS", [128, 16 * 64], F16, kind="ExternalInput")
    tri = nc.dram_tensor("tri", [128, 128], F16, kind="ExternalInput")
    gates = nc.dram_tensor("gates", [16, HPC], F32, kind="ExternalInput")
    eye = nc.dram_tensor("eye", [128, 128], F16, kind="ExternalInput")
    out_ext = nc.dram_tensor("out", [S, OC], F32, kind="ExternalOutput")

    from contextlib import ExitStack
    with tile.TileContext(nc) as tc:
        with (
            tc.tile_pool(name="persist", bufs=1) as persist,
            tc.tile_pool(name="dram", bufs=1, space="DRAM") as dpool,
            tc.tile_pool(name="qst", bufs=4) as qpool,
        ):
            # ---- persistent tiles ------------------------------------------
            kT = persist.tile([128, HPC * S], F16, tag="kT")     # [d, h*t]
            vsb = persist.tile([128, 16 * OC], F16, tag="vsb")   # [t, tc*o]
            akT = persist.tile([128, HPC * 16], F16, tag="akT")  # [d, h*16]
            avg = persist.tile([16, HPC * HD], F16, tag="avg")   # [a, h*d]
            ones = persist.tile([128, 1], F16, tag="ones")
            eyesb = persist.tile([128, 128], F16, tag="eyesb")
            trisb = persist.tile([128, 128], F16, tag="trisb")
            gatesb = persist.tile([16, HPC], F32, tag="gatesb")
            cossb = persist.tile([128, 16 * 64], F16, tag="cossb")
            sinsb = persist.tile([128, 16 * 64], F16, tag="sinsb")
            negC = persist.tile([128, 1], F32, tag="negC")

            nc.gpsimd.memset(ones[:], 1.0)
            nc.gpsimd.memset(negC[:], -9.0)
            nc.sync.dma_start(eyesb[:], eye[:])
            nc.sync.dma_start(trisb[:], tri[:])
            nc.sync.dma_start(gatesb[:], gates[:])
            nc.sync.dma_start(cossb[:], cosS[:])
            nc.sync.dma_start(sinsb[:], sinS[:])

            qstage = {}
            agin = [dpool.tile([OC, QW], F16, tag=f"agin{q}", name=f"agin{q}")
                    for q in range(QB)]
            agout = [dpool.tile([CPG * OC, QW], F16, tag=f"agout{q}",
                                name=f"agout{q}")
                     for q in range(QB)]

            # =============== phase A: QKV projections =======================
            es_a = ExitStack()
            with es_a:
                PA = lambda **kw: es_a.enter_context(tc.tile_pool(**kw))
                wpool = PA(name="wres", bufs=36)
                xpool = PA(name="xin", bufs=8)
                apool = PA(name="asm", bufs=2)
                rpool = PA(name="rot", bufs=2)
                rtpool = PA(name="rt", bufs=6)
                pspool = PA(name="ps1", bufs=3, space="PSUM")
                ptpool = PA(name="pstr", bufs=2, space="PSUM")

                def rope_and_store(proj, tch, asmt):
                    """RoPE on asmt [t,d], transpose into kT/qstage."""
                    a3 = asmt[:].rearrange(
                        "p (h d) -> p h d", h=HPC)[:, :, 0:64]
                    b3 = asmt[:].rearrange(
                        "p (h d) -> p h d", h=HPC)[:, :, 64:128]
                    cos1 = cossb[:].rearrange(
                        "p (c o d) -> p c o d", c=16, o=1)[:, tch]
                    sin1 = sinsb[:].rearrange(
                        "p (c o d) -> p c o d", c=16, o=1)[:, tch]
                    cos3, _ = bass.broadcast_tensor_aps(cos1, a3)
                    sin3, _ = bass.broadcast_tensor_aps(sin1, a3)
                    rot = rpool.tile([128, OC], F16, tag="rot",
                                     name=f"rot{proj}{tch}")
                    ra = rot[:].rearrange(
                        "p (h d) -> p h d", h=HPC)[:, :, 0:64]
                    rb = rot[:].rearrange(
                        "p (h d) -> p h d", h=HPC)[:, :, 64:128]
                    t1 = rtpool.tile([128, HPC * 64], F16, tag="rt",
                                     name=f"rt1{proj}{tch}")
                    t13 = t1[:].rearrange("p (h d) -> p h d", h=HPC)
                    t2 = rtpool.tile([128, HPC * 64], F16, tag="rt",
                                     name=f"rt2{proj}{tch}")
                    t23 = t2[:].rearrange("p (h d) -> p h d", h=HPC)
                    nc.vector.tensor_tensor(t13, a3, cos3, op.mult)
                    nc.vector.tensor_tensor(t23, b3, sin3, op.mult)
                    nc.vector.tensor_tensor(ra, t13, t23, op.subtract)
                    nc.vector.tensor_tensor(t13, a3, sin3, op.mult)
                    nc.vector.tensor_tensor(t23, b3, cos3, op.mult)
                    nc.vector.tensor_tensor(rb, t13, t23, op.add)
                    ptr = ptpool.tile([128, OC], F16, tag="pstr",
                                      name=f"ptr{proj}{tch}")
                    for h in range(HPC):
                        nc.tensor.transpose(
                            ptr[:, h * 128:(h + 1) * 128],
                            rot[:, h * 128:(h + 1) * 128],
                            eyesb[:],
                        )
                    if proj == "k":
                        nc.scalar.activation(
                            kT[:].rearrange(
                                "p (h t) -> p h t",
                                h=HPC)[:, :, tch * 128:(tch + 1) * 128],
                            ptr[:].rearrange("p (h d) -> p h d", h=HPC),
                            Copy,
                        )
                    else:  # q
                        qb = tch // 4
                        off = (tch % 4) * 128
                        nc.scalar.activation(
                            qstage[qb][:].rearrange(
                                "p (h t) -> p h t",
                                h=HPC)[:, :, off:off + 128],
                            ptr[:].rearrange("p (h d) -> p h d", h=HPC),
                            Copy,
                        )

                def proj_chunk(proj, tch, ps):
                    """Post-matmul processing for one [128, OC] psum chunk."""
                    if proj == "v":
                        if tch == 16:  # adapter values -> gated avg
                            for h in range(HPC):
                                nc.vector.tensor_scalar(
                                    avg[0:10, h * HD:(h + 1) * HD],
                                    ps[0:10, h * HD:(h + 1) * HD],
                                    gatesb[0:10, h:h + 1],
                                    None,
                                    op.mult,
                                )
                            return
                        nc.scalar.activation(
                            vsb[:, tch * OC:tch * OC + 512],
                            ps[:, 0:512], Copy)
                        nc.vector.tensor_copy(
                            vsb[:, tch * OC + 512:(tch + 1) * OC],
                            ps[:, 512:1024])
                        return
                    asmt = apool.tile([128, OC], F16, tag="asm",
                                      name=f"as{proj}{tch}")
                    nc.scalar.activation(asmt[:, 0:512], ps[:, 0:512], Copy)
                    nc.vector.tensor_copy(asmt[:, 512:1024], ps[:, 512:1024])
                    if tch == 16:  # adapter chunk (k only): no rope
                        ptr = ptpool.tile([128, OC], F16, tag="pstr",
                                          name="ptrak")
                        for h in range(HPC):
                            nc.tensor.transpose(
                                ptr[:, h * 128:(h + 1) * 128],
                                asmt[:, h * 128:(h + 1) * 128],
                                eyesb[:],
                            )
                        nc.scalar.activation(
                            akT[:].rearrange("p (h a) -> p h a", h=HPC),
                            ptr[:].rearrange(
                                "p (h t) -> p h t", h=HPC)[:, :, 0:16],
                            Copy,
                        )
                        return
                    rope_and_store(proj, tch, asmt)

                def proj_group(proj, wres, chs, first_group=False, wsrc=None):
                    """Group of chunks, m-outer: x tile used 2x then freed."""
                    W = len(chs) * 128
                    c0 = chs[0] * 128
                    psl = [pspool.tile([128, OC], F32, tag="ps1",
                                       name=f"ps{proj}{tch}")
                           for tch in chs]
                    for m in range(NM):
                        if first_group and wsrc is not None:
                            nc.sync.dma_start(
                                wres[m][:], wsrc[m * 128:(m + 1) * 128, :])
                        xt = xpool.tile([128, W], F16, tag="xin",
                                        name=f"x{proj}{chs[0]}_{m}")
                        nc.sync.dma_start(
                            xt[:], xT[m * 128:(m + 1) * 128, c0:c0 + W])
                        for j in range(len(chs)):
                            for half in range(2):
                                nc.tensor.matmul(
                                    psl[j][:, half * 512:(half + 1) * 512],
                                    lhsT=xt[:, j * 128:(j + 1) * 128],
                                    rhs=wres[m][:,
                                                half * 512:(half + 1) * 512],
                                    start=(m == 0),
                                    stop=(m == NM - 1),
                                )
                    for j, tch in enumerate(chs):
                        proj_chunk(proj, tch, psl[j])

                def make_wres(proj):
                    return [wpool.tile([128, OC], F16, tag="wres",
                                       name=f"w{proj}{m}")
                            for m in range(NM)]

                def run_proj(proj, wsrc, chunks):
                    wres = make_wres(proj)
                    groups = [chunks[i:i + 2]
                              for i in range(0, len(chunks), 2)]
                    for gi, chs in enumerate(groups):
                        proj_group(proj, wres, chs,
                                   first_group=(gi == 0), wsrc=wsrc)

                run_proj("k", wkT, list(range(16)) + [16])
                run_proj("v", wvT, list(range(16)) + [16])
                for qb in range(QB):
                    qstage[qb] = qpool.tile([128, HPC * QW], F16,
                                            tag="qstage", name=f"qs{qb}")
                run_proj("q", wqT, list(range(16)))

            # =============== phase B: attention =============================
            es_w = ExitStack()
            es_b = ExitStack()
            with es_w, es_b:
                w2pool = es_w.enter_context(tc.tile_pool(name="w2", bufs=64))
                PB = lambda **kw: es_b.enter_context(tc.tile_pool(**kw))
                prpool = PB(name="probs", bufs=4)
                partpool = PB(name="part", bufs=6)
                appool = PB(name="aprobs", bufs=2)
                recpool = PB(name="rec", bufs=1)
                bcpool = PB(name="bcast", bufs=1)
                ctpool = PB(name="ctmp", bufs=4)
                copool = PB(name="cout", bufs=2)
                pscp = PB(name="psc", bufs=2, space="PSUM")
                ppvp = PB(name="ppv", bufs=3, space="PSUM")
                psmp = PB(name="psm", bufs=1, space="PSUM")

                def attention_block(qb, pending):
                    kk = (qb + 1) * 4
                    npairs = kk // 2
                    qs = qstage[qb]

                    def head_tail(ctx):
                        # deferred softmax-denominator tail for a head
                        sums = psmp.tile([1, QW], F32, tag="psm",
                                         name="sums{}_{}".format(*ctx["id"]))
                        nc.tensor.matmul(
                            sums[:], lhsT=ones[:, 0:1],
                            rhs=ctx["acc"][:], start=True, stop=True)
                        recMA = recpool.tile([1, 2 * QW], F32, tag="rec",
                                             name="rec{}_{}".format(
                                                 *ctx["id"]))
                        nc.vector.reciprocal_approx_fast(
                            recMA[0:1, 0:QW], sums[:])
                        nc.vector.reciprocal_approx_fast(
                            recMA[0:1, QW:2 * QW], ctx["sA"][0:1, :])
                        bcMA = bcpool.tile([128, 2 * QW], F32, tag="bcast",
                                           name="bc{}_{}".format(*ctx["id"]))
                        nc.gpsimd.partition_broadcast(bcMA[:], recMA[:])
                        cqb, ch = ctx["id"]
                        c1 = ctpool.tile([128, QW], F16, tag="ctmp",
                                         name=f"c1{cqb}_{ch}")
                        nc.vector.tensor_tensor(c1[:], ctx["pv"][:],
                                                bcMA[:, 0:QW], op.mult)
                        c2 = ctpool.tile([128, QW], F16, tag="ctmp",
                                         name=f"c2{cqb}_{ch}")
                        nc.vector.tensor_tensor(c2[:], ctx["apv"][:],
                                                bcMA[:, QW:2 * QW], op.mult)
                        c3 = copool.tile([128, QW], F16, tag="cout",
                                         name=f"c3{cqb}_{ch}")
                        nc.vector.tensor_tensor(c3[:], c1[:], c2[:], op.add)
                        nc.sync.dma_start(
                            agin[cqb][ch * 128:(ch + 1) * 128, :], c3[:])

                    for h in range(HPC):
                        q_ap = qs[:, h * QW:(h + 1) * QW]
                        # adapter scores (exp deferred until after pair 0)
                        asc = pscp.tile([10, QW], F32, tag="sc",
                                        name=f"asc{qb}_{h}")
                        nc.tensor.matmul(
                            asc[:], lhsT=akT[:, h * 16:h * 16 + 10],
                            rhs=q_ap, start=True, stop=True)
                        # main causal attention in chunk pairs
                        pv = ppvp.tile([128, QW], F32, tag="pv",
                                       name=f"pv{qb}_{h}")
                        acc = partpool.tile([128, QW], F16, tag="part",
                                            name=f"acc{qb}_{h}")
                        apb = None
                        sA = None
                        for pr in range(npairs):
                            diag = pr >= qb * 2
                            # 2nd diagonal pair: columns < nb are fully
                            # masked for both chunks - skip them everywhere
                            nb = (2 * pr - qb * 4) * 128 if diag else 0
                            sc = pscp.tile([128, 2 * QW], F32, tag="sc",
                                           name=f"sc{qb}_{h}_{pr}")
                            for half in range(2):
                                kc = 2 * pr + half
                                nc.tensor.matmul(
                                    sc[:, half * QW + nb:(half + 1) * QW],
                                    lhsT=kT[:, h * S + kc * 128:
                                            h * S + (kc + 1) * 128],
                                    rhs=q_ap[:, nb:],
                                    start=True, stop=True,
                                )
                            if pr == 0 and pending[0] is not None:
                                # previous head's tail: its sums matmul
                                # slots between our score and pv matmuls
                                head_tail(pending[0])
                                pending[0] = None
                            pb = prpool.tile([128, 2 * QW], F16, tag="probs",
                                             name=f"pb{qb}_{h}_{pr}")
                            if nb:
                                sc3 = sc[:].rearrange(
                                    "p (two c) -> p two c", two=2)[:, :, nb:]
                                pb3 = pb[:].rearrange(
                                    "p (two c) -> p two c", two=2)[:, :, nb:]
                                nc.scalar.activation(pb3, sc3, Exp,
                                                     bias=negC[:, 0:1],
                                                     scale=SCALE)
                            else:
                                nc.scalar.activation(pb[:], sc[:], Exp,
                                                     bias=negC[:, 0:1],
                                                     scale=SCALE)
                            if pr == 0:
                                apb = appool.tile([10, QW], F16,
                                                  tag="aprobs",
                                                  name=f"apb{qb}_{h}")
                                nc.scalar.activation(apb[:], asc[:], Exp,
                                                     bias=negC[0:10, 0:1],
                                                     scale=SCALE)
                                sA = appool.tile([10, QW], F32, tag="sA",
                                                 name=f"sA{qb}_{h}")
                                nc.gpsimd.partition_all_reduce(
                                    sA[:], apb[:], 10,
                                    bass_isa.ReduceOp.add)
                            if diag:
                                # zero the masked band of the hi chunk,
                                # triangle-mask both diagonal blocks
                                nc.vector.tensor_scalar(
                                    pb[:, QW + nb:QW + nb + 128],
                                    pb[:, QW + nb:QW + nb + 128],
                                    0.0, None, op.mult)
                                nc.vector.tensor_tensor(
                                    pb[:, nb:nb + 128],
                                    pb[:, nb:nb + 128],
                                    trisb[:], op.mult)
                                nc.vector.tensor_tensor(
                                    pb[:, QW + nb + 128:QW + nb + 256],
                                    pb[:, QW + nb + 128:QW + nb + 256],
                                    trisb[:], op.mult)
                            part = partpool.tile([128, QW], F16, tag="part",
                                                 name=f"pp{qb}_{h}_{pr}")
                            nc.vector.tensor_tensor(
                                part[:, nb:], pb[:, nb:QW],
                                pb[:, QW + nb:2 * QW], op.add)
                            if pr == 0:
                                nc.vector.tensor_copy(acc[:], part[:])
                            else:
                                nc.vector.tensor_tensor(
                                    acc[:, nb:], acc[:, nb:],
                                    part[:, nb:], op.add)
                            for half in range(2):
                                kc = 2 * pr + half
                                nc.tensor.matmul(
                                    pv[:, nb:],
                                    lhsT=vsb[:, kc * OC + h * HD:
                                             kc * OC + (h + 1) * HD],
                                    rhs=pb[:, half * QW + nb:
                                           (half + 1) * QW],
                                    start=(kc == 0), stop=(kc == kk - 1),
                                )
                        # adapter values
                        apv = ppvp.tile([128, QW], F32, tag="pv",
                                        name=f"apv{qb}_{h}")
                        nc.tensor.matmul(
                            apv[:], lhsT=avg[0:10, h * HD:(h + 1) * HD],
                            rhs=apb[:], start=True, stop=True)
                        pending[0] = {"id": (qb, h), "acc": acc, "sA": sA,
                                      "pv": pv, "apv": apv}
                    # flush the last head before the collective
                    head_tail(pending[0])
                    pending[0] = None
                    nc.gpsimd.collective_compute(
                        "AllGather",
                        op.bypass,
                        replica_groups=REPLICA_GROUPS,
                        ins=[agin[qb][:].opt()],
                        outs=[agout[qb][:].opt()],
                    )

                w2t = {0: [], 1: []}
                pending = [None]
                attention_block(0, pending)
                # wo weight prefetch hides under remaining attention
                for jh in range(2):
                    for m in range(NM):
                        wt = w2pool.tile([128, 512], F16, tag="w2",
                                         name=f"w2_{jh}_{m}")
                        nc.sync.dma_start(
                            wt[:], woT[m * 128:(m + 1) * 128,
                                       jh * 512:(jh + 1) * 512])
                        w2t[jh].append(wt)
                for qb in range(1, QB):
                    attention_block(qb, pending)
                es_b.close()

                # =============== phase C: wo projection =====================
                es_c = ExitStack()
                with es_c:
                    PC = lambda **kw: es_c.enter_context(tc.tile_pool(**kw))
                    agpool = PC(name="agsb", bufs=34)
                    ostpool = PC(name="ost", bufs=2)
                    pwop = PC(name="pwo", bufs=2, space="PSUM")

                    for qb in range(QB):
                        ag = []
                        for i in range(NM):
                            a = agpool.tile([128, QW], F16, tag="agsb",
                                            name=f"ag{qb}_{i}")
                            nc.sync.dma_start(
                                a[:],
                                agout[qb][i * 128:(i + 1) * 128, :])
                            ag.append(a)
                        for jh in range(2):
                            for tsub in range(4):
                                ps = pwop.tile([128, 512], F32, tag="pwo",
                                               name=f"pwo{jh}{qb}{tsub}")
                                for i in range(NM):
                                    nc.tensor.matmul(
                                        ps[:],
                                        lhsT=ag[i][:, tsub * 128:
                                                   (tsub + 1) * 128],
                                        rhs=w2t[jh][i][:],
                                        start=(i == 0), stop=(i == NM - 1),
                                    )
                                st = ostpool.tile([128, 512], F32, tag="ost",
                                                  name=f"st{jh}{qb}{tsub}")
                                nc.scalar.activation(st[:], ps[:], Copy)
                                r0 = qb * QW + tsub * 128
                                nc.sync.dma_start(
                                    out_ext[r0:r0 + 128,
                                            jh * 512:(jh + 1) * 512], st[:])

    nc.compile()
    return nc


# ---------------------------------------------------------------------------
# host-side input prep + execution
# ---------------------------------------------------------------------------

_DEINT = np.concatenate([np.arange(0, 128, 2), np.arange(1, 128, 2)])


def _prep_inputs(x, adapter, wq, wk, wv, wo, gate, freqs_cos, freqs_sin, mask):
    """Build the per-core input maps."""
    perm = np.concatenate([h * HD + _DEINT for h in range(H)])  # deinterleave
    wqp = wq[perm, :]  # permute output dims of wq/wk for rope layout
    wkp = wk[perm, :]

    # cos/sin tables pre-laid for SBUF: [p, c*64] with p = t within chunk
    cosS = np.ascontiguousarray(
        freqs_cos.reshape(16, 128, 64).transpose(1, 0, 2).reshape(128, 1024)
    ).astype(FP16)
    sinS = np.ascontiguousarray(
        freqs_sin.reshape(16, 128, 64).transpose(1, 0, 2).reshape(128, 1024)
    ).astype(FP16)
    # 128x128 causal triangle (transposed): tri[k, q] = exp(mask)[q, k]
    tri = np.ascontiguousarray(
        np.exp(mask[0, 0, 0:128, 0:128]).T).astype(FP16)

    in_maps = []
    for c in range(NCORES):
        g, ci = divmod(c, CPG)
        osl = slice(ci * OC, (ci + 1) * OC)
        xTh = np.zeros((DIM, TAUG), FP16)
        xTh[:, :S] = x[g].T.astype(FP16)
        xTh[:, S:S + ALEN] = adapter[0].T.astype(FP16)
        gatesh = np.zeros((16, HPC), np.float32)
        gatesh[:, :] = gate[0, ci * HPC:(ci + 1) * HPC, 0, 0][None, :]
        in_maps.append({
            "xT": xTh,
            "wqT": np.ascontiguousarray(wqp[osl].T).astype(FP16),
            "wkT": np.ascontiguousarray(wkp[osl].T).astype(FP16),
            "wvT": np.ascontiguousarray(wv[osl].T).astype(FP16),
            "woT": np.ascontiguousarray(wo[osl].T).astype(FP16),
            "cosS": cosS,
            "sinS": sinS,
            "tri": tri,
            "gates": gatesh,
            "eye": np.eye(128, dtype=FP16),
        })
    return in_maps


_NC_CACHE = {}
TRACE = bool(int(os.environ.get("BASS_KERNEL_TRACE", "0")))
LAST_EXEC_NS = None
LAST_RESULTS = None


def kernel(x, adapter, wq, wk, wv, wo, gate, freqs_cos, freqs_sin, mask,
           start_pos=0, **_unused):
    global LAST_EXEC_NS, LAST_RESULTS
    from concourse.bass_utils import run_bass_kernel_spmd

    to_np = lambda a: np.asarray(a)
    x, adapter, wq, wk, wv, wo = map(to_np, (x, adapter, wq, wk, wv, wo))
    gate, freqs_cos, freqs_sin, mask = map(
        to_np, (gate, freqs_cos, freqs_sin, mask))

    if "nc" not in _NC_CACHE:
        _NC_CACHE["nc"] = build_graph()
    nc = _NC_CACHE["nc"]

    in_maps = _prep_inputs(x, adapter, wq, wk, wv, wo, gate,
                           freqs_cos, freqs_sin, mask)
    res = run_bass_kernel_spmd(
        nc, in_maps, core_ids=list(range(NCORES)), trace=TRACE)
    LAST_EXEC_NS = res.exec_time_ns
    LAST_RESULTS = res
    out = np.empty((B, S, DIM), np.float32)
    for c in range(NCORES):
        g, ci = divmod(c, CPG)
        out[g, :, ci * OC:(ci + 1) * OC] = res.results[c]["out"]
    return out


# revision 18
# speedup vs baseline: 1.0330x; 1.0330x over previous
"""Trainium2 Bass kernel for gated-adapter attention (Llama-Adapter style).

Sharding: 2 data-parallel groups of 4 cores (batch dim); within a group the 32
heads are tensor-parallel (8 heads/core).  Each core computes QKV + RoPE +
causal attention (transposed scores) + gated adapter cross attention for its
heads, AllGathers the per-head attention output across its group of 4, then
computes a column shard of the wo projection.  The host reassembles the full
[2, 2048, 4096] output from the 8 per-core shards.

v2 restructure (trace-driven):
 - softmax denominators via fp16 pair-tree on DVE + one ones-matmul per
   (head, qblock) instead of one ones-matmul per key chunk (PE -150us).
 - exp computed on [128,1024] chunk pairs (halves ACT overhead).
 - causal mask via a single 128x128 triangle tile + zero-fill (no mask DMAs).
 - projections run m-outer over [128,256] x tiles (bigger DMA lines, tiny
   x pool); weight DMAs interleaved with first x tiles so PE starts early.
 - q blocks kept in SBUF; wo weights prefetched under attention; wo follows
   immediately after the last attention block.
"""

import math
import os
import sys

import numpy as np

for _p in ("/opt/trn_rl_repo",):
    if os.path.isdir(_p) and _p not in sys.path:
        sys.path.insert(0, _p)

import ml_dtypes  # noqa: E402

import concourse.bass as bass  # noqa: E402
import concourse.mybir as mybir  # noqa: E402
import concourse.tile as tile  # noqa: E402
from concourse import bacc  # noqa: E402
from concourse import bass_isa  # noqa: E402

FP16 = np.float16
F16 = mybir.dt.float16
F32 = mybir.dt.float32

DIM = 4096
S = 2048
B = 2
H = 32
HD = 128
ALEN = 10

NCORES = 8
CPG = 4          # cores per group (group = one batch element)
HPC = 8          # heads per core
OC = HPC * HD    # 1024 output dims per core for q/k/v and for wo columns

TCN = 17         # t-chunks of 128: 16 real + 1 adapter/pad chunk
TAUG = TCN * 128  # 2176
NM = DIM // 128   # 32 contraction chunks
QB = 4           # query blocks
QW = 512         # query block width
SCALE = 1.0 / math.sqrt(HD)

REPLICA_GROUPS = [[0, 1, 2, 3], [4, 5, 6, 7]]

Exp = mybir.ActivationFunctionType.Exp
Copy = mybir.ActivationFunctionType.Copy


def _alu():
    from concourse.alu_op_type import AluOpType
    return AluOpType


def build_graph():
    nc = bacc.Bacc(
        "TRN2",
        target_bir_lowering=False,
        debug=False,
        num_devices=NCORES,
    )
    op = _alu()

    # ---- external I/O ------------------------------------------------------
    xT = nc.dram_tensor("xT", [DIM, TAUG], F16, kind="ExternalInput")
    wqT = nc.dram_tensor("wqT", [DIM, OC], F16, kind="ExternalInput")
    wkT = nc.dram_tensor("wkT", [DIM, OC], F16, kind="ExternalInput")
    wvT = nc.dram_tensor("wvT", [DIM, OC], F16, kind="ExternalInput")
    woT = nc.dram_tensor("woT", [DIM, OC], F16, kind="ExternalInput")
    cosS = nc.dram_tensor("cosS", [128, 16 * 64], F16, kind="ExternalInput")
    sinS = nc.dram_tensor("sinS", [128, 16 * 64], F16, kind="ExternalInput")
    tri = nc.dram_tensor("tri", [128, 128], F16, kind="ExternalInput")
    gates = nc.dram_tensor("gates", [16, HPC], F32, kind="ExternalInput")
    eye = nc.dram_tensor("eye", [128, 128], F16, kind="ExternalInput")
    out_ext = nc.dram_tensor("out", [S, OC], F32, kind="ExternalOutput")

    from contextlib import ExitStack
    with tile.TileContext(nc) as tc:
        with (
            tc.tile_pool(name="persist", bufs=1) as persist,
            tc.tile_pool(name="dram", bufs=1, space="DRAM") as dpool,
            tc.tile_pool(name="qst", bufs=4) as qpool,
        ):
            # ---- persistent tiles ------------------------------------------
            kT = persist.tile([128, HPC * S], F16, tag="kT")     # [d, h*t]
            vsb = persist.tile([128, 16 * OC], F16, tag="vsb")   # [t, tc*o]
            akT = persist.tile([128, HPC * 16], F16, tag="akT")  # [d, h*16]
            avg = persist.tile([16, HPC * HD], F16, tag="avg")   # [a, h*d]
            ones = persist.tile([128, 1], F16, tag="ones")
            eyesb = persist.tile([128, 128], F16, tag="eyesb")
            trisb = persist.tile([128, 128], F16, tag="trisb")
            gatesb = persist.tile([16, HPC], F32, tag="gatesb")
            cossb = persist.tile([128, 16 * 64], F16, tag="cossb")
            sinsb = persist.tile([128, 16 * 64], F16, tag="sinsb")
            negC = persist.tile([128, 1], F32, tag="negC")

            nc.gpsimd.memset(ones[:], 1.0)
            nc.gpsimd.memset(negC[:], -9.0)
            nc.sync.dma_start(eyesb[:], eye[:])
            nc.sync.dma_start(trisb[:], tri[:])
            nc.sync.dma_start(gatesb[:], gates[:])
            nc.sync.dma_start(cossb[:], cosS[:])
            nc.sync.dma_start(sinsb[:], sinS[:])

            qstage = {}
            agin = [dpool.tile([OC, QW], F16, tag=f"agin{q}", name=f"agin{q}")
                    for q in range(QB)]
            agout = [dpool.tile([CPG * OC, QW], F16, tag=f"agout{q}",
                                name=f"agout{q}")
                     for q in range(QB)]

            # =============== phase A: QKV projections =======================
            es_a = ExitStack()
            with es_a:
                PA = lambda **kw: es_a.enter_context(tc.tile_pool(**kw))
                wpool = PA(name="wres", bufs=40)
                xpool = PA(name="xin", bufs=8)
                apool = PA(name="asm", bufs=2)
                rpool = PA(name="rot", bufs=2)
                rtpool = PA(name="rt", bufs=6)
                pspool = PA(name="ps1", bufs=3, space="PSUM")
                ptpool = PA(name="pstr", bufs=2, space="PSUM")

                def rope_and_store(proj, tch, asmt):
                    """RoPE on asmt [t,d], transpose into kT/qstage."""
                    a3 = asmt[:].rearrange(
                        "p (h d) -> p h d", h=HPC)[:, :, 0:64]
                    b3 = asmt[:].rearrange(
                        "p (h d) -> p h d", h=HPC)[:, :, 64:128]
                    cos1 = cossb[:].rearrange(
                        "p (c o d) -> p c o d", c=16, o=1)[:, tch]
                    sin1 = sinsb[:].rearrange(
                        "p (c o d) -> p c o d", c=16, o=1)[:, tch]
                    cos3, _ = bass.broadcast_tensor_aps(cos1, a3)
                    sin3, _ = bass.broadcast_tensor_aps(sin1, a3)
                    rot = rpool.tile([128, OC], F16, tag="rot",
                                     name=f"rot{proj}{tch}")
                    ra = rot[:].rearrange(
                        "p (h d) -> p h d", h=HPC)[:, :, 0:64]
                    rb = rot[:].rearrange(
                        "p (h d) -> p h d", h=HPC)[:, :, 64:128]
                    t1 = rtpool.tile([128, HPC * 64], F16, tag="rt",
                                     name=f"rt1{proj}{tch}")
                    t13 = t1[:].rearrange("p (h d) -> p h d", h=HPC)
                    t2 = rtpool.tile([128, HPC * 64], F16, tag="rt",
                                     name=f"rt2{proj}{tch}")
                    t23 = t2[:].rearrange("p (h d) -> p h d", h=HPC)
                    nc.vector.tensor_tensor(t13, a3, cos3, op.mult)
                    nc.vector.tensor_tensor(t23, b3, sin3, op.mult)
                    nc.vector.tensor_tensor(ra, t13, t23, op.subtract)
                    nc.vector.tensor_tensor(t13, a3, sin3, op.mult)
                    nc.vector.tensor_tensor(t23, b3, cos3, op.mult)
                    nc.vector.tensor_tensor(rb, t13, t23, op.add)
                    ptr = ptpool.tile([128, OC], F16, tag="pstr",
                                      name=f"ptr{proj}{tch}")
                    for h in range(HPC):
                        nc.tensor.transpose(
                            ptr[:, h * 128:(h + 1) * 128],
                            rot[:, h * 128:(h + 1) * 128],
                            eyesb[:],
                        )
                    if proj == "k":
                        nc.scalar.activation(
                            kT[:].rearrange(
                                "p (h t) -> p h t",
                                h=HPC)[:, :, tch * 128:(tch + 1) * 128],
                            ptr[:].rearrange("p (h d) -> p h d", h=HPC),
                            Copy,
                        )
                    else:  # q
                        qb = tch // 4
                        off = (tch % 4) * 128
                        nc.scalar.activation(
                            qstage[qb][:].rearrange(
                                "p (h t) -> p h t",
                                h=HPC)[:, :, off:off + 128],
                            ptr[:].rearrange("p (h d) -> p h d", h=HPC),
                            Copy,
                        )

                def proj_chunk(proj, tch, ps):
                    """Post-matmul processing for one [128, OC] psum chunk."""
                    if proj == "v":
                        if tch == 16:  # adapter values -> gated avg
                            for h in range(HPC):
                                nc.vector.tensor_scalar(
                                    avg[0:10, h * HD:(h + 1) * HD],
                                    ps[0:10, h * HD:(h + 1) * HD],
                                    gatesb[0:10, h:h + 1],
                                    None,
                                    op.mult,
                                )
                            return
                        nc.scalar.activation(
                            vsb[:, tch * OC:tch * OC + 512],
                            ps[:, 0:512], Copy)
                        nc.vector.tensor_copy(
                            vsb[:, tch * OC + 512:(tch + 1) * OC],
                            ps[:, 512:1024])
                        return
                    asmt = apool.tile([128, OC], F16, tag="asm",
                                      name=f"as{proj}{tch}")
                    nc.scalar.activation(asmt[:, 0:512], ps[:, 0:512], Copy)
                    nc.vector.tensor_copy(asmt[:, 512:1024], ps[:, 512:1024])
                    if tch == 16:  # adapter chunk (k only): no rope
                        ptr = ptpool.tile([128, OC], F16, tag="pstr",
                                          name="ptrak")
                        for h in range(HPC):
                            nc.tensor.transpose(
                                ptr[:, h * 128:(h + 1) * 128],
                                asmt[:, h * 128:(h + 1) * 128],
                                eyesb[:],
                            )
                        nc.scalar.activation(
                            akT[:].rearrange("p (h a) -> p h a", h=HPC),
                            ptr[:].rearrange(
                                "p (h t) -> p h t", h=HPC)[:, :, 0:16],
                            Copy,
                        )
                        return
                    rope_and_store(proj, tch, asmt)

                def proj_group(proj, wres, chs, first_group=False, wsrc=None):
                    """Group of chunks, m-outer: x tile used 2x then freed."""
                    W = len(chs) * 128
                    c0 = chs[0] * 128
                    psl = [pspool.tile([128, OC], F32, tag="ps1",
                                       name=f"ps{proj}{tch}")
                           for tch in chs]
                    for m in range(NM):
                        if first_group and wsrc is not None:
                            nc.sync.dma_start(
                                wres[m][:], wsrc[m * 128:(m + 1) * 128, :])
                        xt = xpool.tile([128, W], F16, tag="xin",
                                        name=f"x{proj}{chs[0]}_{m}")
                        nc.sync.dma_start(
                            xt[:], xT[m * 128:(m + 1) * 128, c0:c0 + W])
                        for j in range(len(chs)):
                            for half in range(2):
                                nc.tensor.matmul(
                                    psl[j][:, half * 512:(half + 1) * 512],
                                    lhsT=xt[:, j * 128:(j + 1) * 128],
                                    rhs=wres[m][:,
                                                half * 512:(half + 1) * 512],
                                    start=(m == 0),
                                    stop=(m == NM - 1),
                                )
                    for j, tch in enumerate(chs):
                        proj_chunk(proj, tch, psl[j])

                def make_wres(proj):
                    return [wpool.tile([128, OC], F16, tag="wres",
                                       name=f"w{proj}{m}")
                            for m in range(NM)]

                def run_proj(proj, wsrc, chunks):
                    wres = make_wres(proj)
                    groups = [chunks[i:i + 2]
                              for i in range(0, len(chunks), 2)]
                    for gi, chs in enumerate(groups):
                        proj_group(proj, wres, chs,
                                   first_group=(gi == 0), wsrc=wsrc)

                run_proj("k", wkT, list(range(16)) + [16])
                run_proj("v", wvT, list(range(16)) + [16])
                for qb in range(QB):
                    qstage[qb] = qpool.tile([128, HPC * QW], F16,
                                            tag="qstage", name=f"qs{qb}")
                run_proj("q", wqT, list(range(16)))

            # =============== phase B: attention =============================
            es_w = ExitStack()
            es_b = ExitStack()
            with es_w, es_b:
                w2pool = es_w.enter_context(tc.tile_pool(name="w2", bufs=64))
                PB = lambda **kw: es_b.enter_context(tc.tile_pool(**kw))
                prpool = PB(name="probs", bufs=3)
                partpool = PB(name="part", bufs=10)
                appool = PB(name="aprobs", bufs=2)
                recpool = PB(name="rec", bufs=1)
                bcpool = PB(name="bcast", bufs=1)
                ctpool = PB(name="ctmp", bufs=4)
                copool = PB(name="cout", bufs=2)
                pscp = PB(name="psc", bufs=2, space="PSUM")
                ppvp = PB(name="ppv", bufs=3, space="PSUM")
                psmp = PB(name="psm", bufs=1, space="PSUM")

                def attention_block(qb, pending):
                    kk = (qb + 1) * 4
                    npairs = kk // 2
                    qs = qstage[qb]

                    def head_tail(ctx):
                        # deferred softmax-denominator tail for a head
                        sums = psmp.tile([1, QW], F32, tag="psm",
                                         name="sums{}_{}".format(*ctx["id"]))
                        nc.tensor.matmul(
                            sums[:], lhsT=ones[:, 0:1],
                            rhs=ctx["acc"][:], start=True, stop=True)
                        recMA = recpool.tile([1, 2 * QW], F32, tag="rec",
                                             name="rec{}_{}".format(
                                                 *ctx["id"]))
                        nc.vector.reciprocal_approx_fast(
                            recMA[0:1, 0:QW], sums[:])
                        nc.vector.reciprocal_approx_fast(
                            recMA[0:1, QW:2 * QW], ctx["sA"][0:1, :])
                        bcMA = bcpool.tile([128, 2 * QW], F32, tag="bcast",
                                           name="bc{}_{}".format(*ctx["id"]))
                        nc.gpsimd.partition_broadcast(bcMA[:], recMA[:])
                        cqb, ch = ctx["id"]
                        c1 = ctpool.tile([128, QW], F16, tag="ctmp",
                                         name=f"c1{cqb}_{ch}")
                        nc.vector.tensor_tensor(c1[:], ctx["pv"][:],
                                                bcMA[:, 0:QW], op.mult)
                        c2 = ctpool.tile([128, QW], F16, tag="ctmp",
                                         name=f"c2{cqb}_{ch}")
                        nc.vector.tensor_tensor(c2[:], ctx["apv"][:],
                                                bcMA[:, QW:2 * QW], op.mult)
                        c3 = copool.tile([128, QW], F16, tag="cout",
                                         name=f"c3{cqb}_{ch}")
                        nc.vector.tensor_tensor(c3[:], c1[:], c2[:], op.add)
                        nc.sync.dma_start(
                            agin[cqb][ch * 128:(ch + 1) * 128, :], c3[:])

                    for h in range(HPC):
                        q_ap = qs[:, h * QW:(h + 1) * QW]
                        # adapter scores early (overlap with main loop)
                        asc = pscp.tile([10, QW], F32, tag="sc",
                                        name=f"asc{qb}_{h}")
                        nc.tensor.matmul(
                            asc[:], lhsT=akT[:, h * 16:h * 16 + 10],
                            rhs=q_ap, start=True, stop=True)
                        apb = appool.tile([10, QW], F16, tag="aprobs",
                                          name=f"apb{qb}_{h}")
                        nc.scalar.activation(apb[:], asc[:], Exp,
                                             bias=negC[0:10, 0:1],
                                             scale=SCALE)
                        sA = appool.tile([10, QW], F32, tag="sA",
                                         name=f"sA{qb}_{h}")
                        nc.gpsimd.partition_all_reduce(
                            sA[:], apb[:], 10, bass_isa.ReduceOp.add)
                        # main causal attention in chunk pairs
                        pv = ppvp.tile([128, QW], F32, tag="pv",
                                       name=f"pv{qb}_{h}")
                        acc = None
                        for pr in range(npairs):
                            sc = pscp.tile([128, 2 * QW], F32, tag="sc",
                                           name=f"sc{qb}_{h}_{pr}")
                            for half in range(2):
                                kc = 2 * pr + half
                                nc.tensor.matmul(
                                    sc[:, half * QW:(half + 1) * QW],
                                    lhsT=kT[:, h * S + kc * 128:
                                            h * S + (kc + 1) * 128],
                                    rhs=q_ap,
                                    start=True, stop=True,
                                )
                            if pr == 0 and pending[0] is not None:
                                # previous head's tail: its sums matmul
                                # slots between our score and pv matmuls
                                head_tail(pending[0])
                                pending[0] = None
                            pb = prpool.tile([128, 2 * QW], F16, tag="probs",
                                             name=f"pb{qb}_{h}_{pr}")
                            nc.scalar.activation(pb[:], sc[:], Exp,
                                                 bias=negC[:, 0:1],
                                                 scale=SCALE)
                            if pr >= qb * 2:  # diagonal pair: causal mask
                                for half in range(2):
                                    dk = 2 * pr + half - qb * 4
                                    o = half * QW
                                    if dk > 0:
                                        nc.vector.tensor_scalar(
                                            pb[:, o:o + dk * 128],
                                            pb[:, o:o + dk * 128],
                                            0.0, None, op.mult)
                                    nc.vector.tensor_tensor(
                                        pb[:, o + dk * 128:
                                           o + (dk + 1) * 128],
                                        pb[:, o + dk * 128:
                                           o + (dk + 1) * 128],
                                        trisb[:], op.mult)
                            part = partpool.tile([128, QW], F16, tag="part",
                                                 name=f"pp{qb}_{h}_{pr}")
                            nc.vector.tensor_tensor(part[:], pb[:, 0:QW],
                                                    pb[:, QW:2 * QW], op.add)
                            if acc is None:
                                acc = part
                            else:
                                nacc = partpool.tile(
                                    [128, QW], F16, tag="part",
                                    name=f"acc{qb}_{h}_{pr}")
                                nc.vector.tensor_tensor(
                                    nacc[:], acc[:], part[:], op.add)
                                acc = nacc
                            for half in range(2):
                                kc = 2 * pr + half
                                nc.tensor.matmul(
                                    pv[:],
                                    lhsT=vsb[:, kc * OC + h * HD:
                                             kc * OC + (h + 1) * HD],
                                    rhs=pb[:, half * QW:(half + 1) * QW],
                                    start=(kc == 0), stop=(kc == kk - 1),
                                )
                        # adapter values
                        apv = ppvp.tile([128, QW], F32, tag="pv",
                                        name=f"apv{qb}_{h}")
                        nc.tensor.matmul(
                            apv[:], lhsT=avg[0:10, h * HD:(h + 1) * HD],
                            rhs=apb[:], start=True, stop=True)
                        pending[0] = {"id": (qb, h), "acc": acc, "sA": sA,
                                      "pv": pv, "apv": apv}
                    # flush the last head before the collective
                    head_tail(pending[0])
                    pending[0] = None
                    nc.gpsimd.collective_compute(
                        "AllGather",
                        op.bypass,
                        replica_groups=REPLICA_GROUPS,
                        ins=[agin[qb][:].opt()],
                        outs=[agout[qb][:].opt()],
                    )

                w2t = {0: [], 1: []}
                pending = [None]
                attention_block(0, pending)
                # wo weight prefetch hides under remaining attention
                for jh in range(2):
                    for m in range(NM):
                        wt = w2pool.tile([128, 512], F16, tag="w2",
                                         name=f"w2_{jh}_{m}")
                        nc.sync.dma_start(
                            wt[:], woT[m * 128:(m + 1) * 128,
                                       jh * 512:(jh + 1) * 512])
                        w2t[jh].append(wt)
                for qb in range(1, QB):
                    attention_block(qb, pending)
                es_b.close()

                # =============== phase C: wo projection =====================
                es_c = ExitStack()
                with es_c:
                    PC = lambda **kw: es_c.enter_context(tc.tile_pool(**kw))
                    agpool = PC(name="agsb", bufs=34)
                    ostpool = PC(name="ost", bufs=2)
                    pwop = PC(name="pwo", bufs=2, space="PSUM")

                    for qb in range(QB):
                        ag = []
                        for i in range(NM):
                            a = agpool.tile([128, QW], F16, tag="agsb",
                                            name=f"ag{qb}_{i}")
                            nc.sync.dma_start(
                                a[:],
                                agout[qb][i * 128:(i + 1) * 128, :])
                            ag.append(a)
                        for jh in range(2):
                            for tsub in range(4):
                                ps = pwop.tile([128, 512], F32, tag="pwo",
                                               name=f"pwo{jh}{qb}{tsub}")
                                for i in range(NM):
                                    nc.tensor.matmul(
                                        ps[:],
                                        lhsT=ag[i][:, tsub * 128:
                                                   (tsub + 1) * 128],
                                        rhs=w2t[jh][i][:],
                                        start=(i == 0), stop=(i == NM - 1),
                                    )
                                st = ostpool.tile([128, 512], F32, tag="ost",
                                                  name=f"st{jh}{qb}{tsub}")
                                nc.scalar.activation(st[:], ps[:], Copy)
                                r0 = qb * QW + tsub * 128
                                nc.sync.dma_start(
                                    out_ext[r0:r0 + 128,
                                            jh * 512:(jh + 1) * 512], st[:])

    nc.compile()
    return nc


# ---------------------------------------------------------------------------
# host-side input prep + execution
# ---------------------------------------------------------------------------

_DEINT = np.concatenate([np.arange(0, 128, 2), np.arange(1, 128, 2)])


def _prep_inputs(x, adapter, wq, wk, wv, wo, gate, freqs_cos, freqs_sin, mask):
    """Build the per-core input maps."""
    perm = np.concatenate([h * HD + _DEINT for h in range(H)])  # deinterleave
    wqp = wq[perm, :]  # permute output dims of wq/wk for rope layout
    wkp = wk[perm, :]

    # cos/sin tables pre-laid for SBUF: [p, c*64] with p = t within chunk
    cosS = np.ascontiguousarray(
        freqs_cos.reshape(16, 128, 64).transpose(1, 0, 2).reshape(128, 1024)
    ).astype(FP16)
    sinS = np.ascontiguousarray(
        freqs_sin.reshape(16, 128, 64).transpose(1, 0, 2).reshape(128, 1024)
    ).astype(FP16)
    # 128x128 causal triangle (transposed): tri[k, q] = exp(mask)[q, k]
    tri = np.ascontiguousarray(
        np.exp(mask[0, 0, 0:128, 0:128]).T).astype(FP16)

    in_maps = []
    for c in range(NCORES):
        g, ci = divmod(c, CPG)
        osl = slice(ci * OC, (ci + 1) * OC)
        xTh = np.zeros((DIM, TAUG), FP16)
        xTh[:, :S] = x[g].T.astype(FP16)
        xTh[:, S:S + ALEN] = adapter[0].T.astype(FP16)
        gatesh = np.zeros((16, HPC), np.float32)
        gatesh[:, :] = gate[0, ci * HPC:(ci + 1) * HPC, 0, 0][None, :]
        in_maps.append({
            "xT": xTh,
            "wqT": np.ascontiguousarray(wqp[osl].T).astype(FP16),
            "wkT": np.ascontiguousarray(wkp[osl].T).astype(FP16),
            "wvT": np.ascontiguousarray(wv[osl].T).astype(FP16),
            "woT": np.ascontiguousarray(wo[osl].T).astype(FP16),
            "cosS": cosS,
            "sinS": sinS,
            "tri": tri,
            "gates": gatesh,
            "eye": np.eye(128, dtype=FP16),
        })
    return in_maps


_NC_CACHE = {}
TRACE = bool(int(os.environ.get("BASS_KERNEL_TRACE", "0")))
LAST_EXEC_NS = None
LAST_RESULTS = None


def kernel(x, adapter, wq, wk, wv, wo, gate, freqs_cos, freqs_sin, mask,
           start_pos=0, **_unused):
    global LAST_EXEC_NS, LAST_RESULTS
    from concourse.bass_utils import run_bass_kernel_spmd

    to_np = lambda a: np.asarray(a)
    x, adapter, wq, wk, wv, wo = map(to_np, (x, adapter, wq, wk, wv, wo))
    gate, freqs_cos, freqs_sin, mask = map(
        to_np, (gate, freqs_cos, freqs_sin, mask))

    if "nc" not in _NC_CACHE:
        _NC_CACHE["nc"] = build_graph()
    nc = _NC_CACHE["nc"]

    in_maps = _prep_inputs(x, adapter, wq, wk, wv, wo, gate,
                           freqs_cos, freqs_sin, mask)
    res = run_bass_kernel_spmd(
        nc, in_maps, core_ids=list(range(NCORES)), trace=TRACE)
    LAST_EXEC_NS = res.exec_time_ns
    LAST_RESULTS = res
    out = np.empty((B, S, DIM), np.float32)
    for c in range(NCORES):
        g, ci = divmod(c, CPG)
        out[g, :, ci * OC:(ci + 1) * OC] = res.results[c]["out"]
    return out
